# revision 4
# baseline (speedup 1.0000x reference)
"""Multi-head causal attention (B=2, S=2048, D=2048, H=16) on 8 trn2 cores.

Sharding: tensor-parallel over heads. Core c owns heads 2c, 2c+1 (256
features of q/k/v). Each core computes its heads' QKV projections (+RoPE),
causal attention, and a partial output through its slice of wo. The 8
partial outputs are summed on the host (the "all-reduce").

Layouts on device (per core):
  qT, kT: [hd=128 partitions, head, token]  (features on partitions, f32r)
          head dims permuted (evens then odds) via host-permuted wq/wk rows
          so RoPE pairs sit in partition halves.
  v:      [token partitions, token-chunk, feature]  (natural, f32r)
  scoresT chunk = matmul(lhsT=kT chunk, rhs=qT tile) -> [kt 128, q 512] PSUM
  probsT = exp(scoresT/sqrt(hd)) (no max subtraction: scores are O(1)-bounded)
  attnT accum = matmul(lhsT=v chunk, rhs=probsT) -> [hd, q] PSUM
  denom = matmul(lhsT=ones, rhs=probsT) -> [1, q] PSUM; normalize at eviction.
  out rows = matmul(lhsT=attnT t-sub, rhs=woT) -> [t 128, j] PSUM -> DRAM.
"""

import math

import numpy as np

B = 2
S = 2048
D = 2048
H = 16
HD = 128
NCORES = 8
FPC = D // NCORES          # 256 features (2 heads) per core
P = 128
ND = D // P                # 16 contraction chunks
TT_N = 512                 # token tile (matmul free dim)
NTT = S // TT_N            # 4 token tiles per batch
NKT = S // P               # 16 key chunks per batch
SCALE = 1.0 / math.sqrt(HD)

_CACHE = {}


def _build_nc():
    import concourse.bass as bass  # noqa: F401
    from concourse import bacc
    import concourse.mybir as mybir
    import concourse.tile as tile

    f32 = mybir.dt.float32
    f32r = mybir.dt.float32r

    nc = bacc.Bacc(None, target_bir_lowering=False)

    xT = nc.dram_tensor("xT", [D, B * S], f32r, kind="ExternalInput")
    wqT = nc.dram_tensor("wqT", [D, FPC], f32r, kind="ExternalInput")
    wkT = nc.dram_tensor("wkT", [D, FPC], f32r, kind="ExternalInput")
    wvT = nc.dram_tensor("wvT", [D, FPC], f32r, kind="ExternalInput")
    woT = nc.dram_tensor("woT", [FPC, D], f32r, kind="ExternalInput")
    cosS = nc.dram_tensor("cosS", [P, S], f32, kind="ExternalInput")
    sinS = nc.dram_tensor("sinS", [P, S], f32, kind="ExternalInput")
    masks = nc.dram_tensor("masks", [P, 4, TT_N], f32r, kind="ExternalInput")
    onesd = nc.dram_tensor("onesd", [P, 1], f32r, kind="ExternalInput")
    outp = nc.dram_tensor("outp", [B * S, D], f32, kind="ExternalOutput")

    with tile.TileContext(nc) as tc:
        with (
            tc.tile_pool(name="res", bufs=1) as res,
            tc.tile_pool(name="xp", bufs=6) as xp,
            tc.tile_pool(name="aTp", bufs=2) as aTp,
            tc.tile_pool(name="probsp", bufs=4) as probsp,
            tc.tile_pool(name="ropep", bufs=2) as ropep,
            tc.tile_pool(name="csp", bufs=2) as csp,
            tc.tile_pool(name="recipp", bufs=2) as recipp,
            tc.tile_pool(name="rbp", bufs=2) as rbp,
            tc.tile_pool(name="ostp", bufs=3) as ostp,
            tc.tile_pool(name="ps_big", bufs=4, space="PSUM") as ps_big,
            tc.tile_pool(name="ps_v", bufs=4, space="PSUM") as ps_v,
        ):
            # resident tensors
            wq_sb = res.tile([P, ND, FPC], f32r)
            wk_sb = res.tile([P, ND, FPC], f32r)
            wv_sb = res.tile([P, ND, FPC], f32r)
            wo_sb = res.tile([P, 2, D], f32r)
            mask_sb = res.tile([P, 4, TT_N], f32r)
            ones_sb = res.tile([P, 1], f32r)
            qT_sb = res.tile([P, 2, S], f32r)
            kT_sb = res.tile([P, 2, S], f32r)
            v_sb = res.tile([P, NKT, FPC], f32r)

            nc.sync.dma_start(out=wq_sb[:], in_=wqT.rearrange("(a p) m -> p a m", p=P))
            nc.sync.dma_start(out=wk_sb[:], in_=wkT.rearrange("(a p) m -> p a m", p=P))
            nc.sync.dma_start(out=wv_sb[:], in_=wvT.rearrange("(a p) m -> p a m", p=P))
            nc.sync.dma_start(out=wo_sb[:], in_=woT.rearrange("(a p) m -> p a m", p=P))
            nc.sync.dma_start(out=mask_sb[:], in_=masks[:])
            nc.sync.dma_start(out=ones_sb[:], in_=onesd[:])

            for b in range(B):
                t0g = b * S

                # ---------------- QKV projections ----------------
                for tt in range(NTT):
                    tsl = slice(tt * TT_N, (tt + 1) * TT_N)
                    gsl = slice(t0g + tt * TT_N, t0g + (tt + 1) * TT_N)

                    qk_ps = [ps_big.tile([P, TT_N], f32, name="big")
                             for _ in range(4)]
                    v_ps = [ps_v.tile([P, FPC], f32, name="vps")
                            for _ in range(TT_N // P)]

                    for d in range(ND):
                        xt = xp.tile([P, TT_N], f32r, name="xt")
                        nc.sync.dma_start(
                            out=xt[:], in_=xT[d * P:(d + 1) * P, gsl]
                        )
                        for fc in range(2):
                            nc.tensor.matmul(
                                qk_ps[fc][:],
                                wq_sb[:, d, fc * P:(fc + 1) * P],
                                xt[:],
                                start=(d == 0), stop=(d == ND - 1),
                            )
                            nc.tensor.matmul(
                                qk_ps[2 + fc][:],
                                wk_sb[:, d, fc * P:(fc + 1) * P],
                                xt[:],
                                start=(d == 0), stop=(d == ND - 1),
                            )
                        for sub in range(TT_N // P):
                            nc.tensor.matmul(
                                v_ps[sub][:],
                                xt[:, sub * P:(sub + 1) * P],
                                wv_sb[:, d, :],
                                start=(d == 0), stop=(d == ND - 1),
                            )
                    for sub in range(TT_N // P):
                        nc.scalar.copy(v_sb[:, tt * 4 + sub, :], v_ps[sub][:])

                    # RoPE + eviction for q,k (features on partitions:
                    # rows 0:64 = even head dims (xr), 64:128 = odd (xi))
                    cct = csp.tile([P, TT_N], f32, name="cct")
                    sst = csp.tile([P, TT_N], f32, name="sst")
                    nc.sync.dma_start(out=cct[:], in_=cosS[:, tsl])
                    nc.sync.dma_start(out=sst[:], in_=sinS[:, tsl])

                    for i, dst in ((0, qT_sb), (1, qT_sb), (2, kT_sb), (3, kT_sb)):
                        fc = i % 2
                        qp = qk_ps[i]
                        p1 = ropep.tile([P, TT_N], f32, name="p1")
                        p2 = ropep.tile([P, TT_N], f32, name="p2")
                        # p1 = [xr*c ; xi*c]
                        nc.vector.tensor_tensor(
                            out=p1[:], in0=qp[:], in1=cct[:],
                            op=mybir.AluOpType.mult,
                        )
                        # p2 top = xi*s (psum base 64 x sbuf base 0)
                        nc.vector.tensor_tensor(
                            out=p2[0:64, :], in0=qp[64:128, :], in1=sst[0:64, :],
                            op=mybir.AluOpType.mult,
                        )
                        # p2 bot = xr*s (psum base 0 x sbuf base 64)
                        nc.vector.tensor_tensor(
                            out=p2[64:128, :], in0=qp[0:64, :], in1=sst[64:128, :],
                            op=mybir.AluOpType.mult,
                        )
                        nc.vector.tensor_tensor(
                            out=dst[0:64, fc, tsl], in0=p1[0:64, :], in1=p2[0:64, :],
                            op=mybir.AluOpType.subtract,
                        )
                        nc.vector.tensor_tensor(
                            out=dst[64:128, fc, tsl], in0=p1[64:128, :],
                            in1=p2[64:128, :], op=mybir.AluOpType.add,
                        )

                # ---------------- attention + wo, per q-tile ----------------
                for qt in range(NTT):
                    qsl = slice(qt * TT_N, (qt + 1) * TT_N)
                    nkt = 4 * qt + 4
                    aT = aTp.tile([P, 2, TT_N], f32r, name="aT")
                    for h in range(2):
                        a_ps = ps_big.tile([P, TT_N], f32, name="big")
                        d_ps = ps_big.tile([1, TT_N], f32, name="big")
                        for kt in range(nkt):
                            s_ps = ps_big.tile([P, TT_N], f32, name="big")
                            nc.tensor.matmul(
                                s_ps[:],
                                kT_sb[:, h, kt * P:(kt + 1) * P],
                                qT_sb[:, h, qsl],
                                start=True, stop=True,
                            )
                            probs = probsp.tile([P, TT_N], f32r, name="probs")
                            nc.scalar.activation(
                                probs[:], s_ps[:],
                                mybir.ActivationFunctionType.Exp,
                                scale=SCALE,
                            )
                            if kt >= 4 * qt:  # diagonal band: causal mask
                                o = kt - 4 * qt
                                nc.vector.tensor_tensor(
                                    out=probs[:], in0=probs[:],
                                    in1=mask_sb[:, o, :],
                                    op=mybir.AluOpType.mult,
                                )
                            nc.tensor.matmul(
                                a_ps[:],
                                v_sb[:, kt, h * P:(h + 1) * P],
                                probs[:],
                                start=(kt == 0), stop=(kt == nkt - 1),
                            )
                            nc.tensor.matmul(
                                d_ps[:],
                                ones_sb[:],
                                probs[:],
                                start=(kt == 0), stop=(kt == nkt - 1),
                            )
                        recip = recipp.tile([1, TT_N], f32, name="recip")
                        nc.vector.reciprocal(recip[:], d_ps[:])
                        rb = rbp.tile([P, TT_N], f32, name="rb")
                        nc.gpsimd.partition_broadcast(rb[:], recip[:])
                        nc.vector.tensor_tensor(
                            out=aT[:, h, :], in0=a_ps[:], in1=rb[:],
                            op=mybir.AluOpType.mult,
                        )

                    # wo for these 512 tokens
                    for ts in range(TT_N // P):
                        trow = t0g + qt * TT_N + ts * P
                        for jc in range(D // TT_N):
                            o_ps = ps_big.tile([P, TT_N], f32, name="big")
                            for h in range(2):
                                nc.tensor.matmul(
                                    o_ps[:],
                                    aT[:, h, ts * P:(ts + 1) * P],
                                    wo_sb[:, h, jc * TT_N:(jc + 1) * TT_N],
                                    start=(h == 0), stop=(h == 1),
                                )
                            ost = ostp.tile([P, TT_N], f32, name="ost")
                            nc.scalar.copy(ost[:], o_ps[:])
                            nc.sync.dma_start(
                                out=outp[trow:trow + P, jc * TT_N:(jc + 1) * TT_N],
                                in_=ost[:],
                            )
    nc.compile()
    return nc


def _host_prep(x, wq, wk, wv, wo):
    x = np.asarray(x, dtype=np.float32)
    wq = np.asarray(wq, dtype=np.float32)
    wk = np.asarray(wk, dtype=np.float32)
    wv = np.asarray(wv, dtype=np.float32)
    wo = np.asarray(wo, dtype=np.float32)

    xT = np.ascontiguousarray(x.reshape(B * S, D).T)  # [D, B*S]

    # permute q/k head dims: per head, even dims then odd dims
    perm = np.concatenate(
        [h * HD + np.concatenate([np.arange(0, HD, 2), np.arange(1, HD, 2)])
         for h in range(H)]
    )
    wq_p = wq[perm]
    wk_p = wk[perm]

    # rope tables, stacked twice on the partition axis
    inv_freq = 1.0 / (10000.0 ** (np.arange(0, HD, 2, dtype=np.float64) / HD))
    t = np.arange(S, dtype=np.float64)
    freqs = t[:, None] * inv_freq[None, :]            # [S, 64]
    cosT = np.cos(freqs).T.astype(np.float32)         # [64, S]
    sinT = np.sin(freqs).T.astype(np.float32)
    cosS = np.ascontiguousarray(np.vstack([cosT, cosT]))  # [128, S]
    sinS = np.ascontiguousarray(np.vstack([sinT, sinT]))

    # causal masks for the 4 diagonal-band offsets
    pidx = np.arange(P)[:, None]
    qidx = np.arange(TT_N)[None, :]
    m = np.stack(
        [(qidx >= o * P + pidx).astype(np.float32) for o in range(4)], axis=1
    )  # [128, 4, 512]
    m = np.ascontiguousarray(m)

    ones = np.ones((P, 1), dtype=np.float32)

    in_maps = []
    for c in range(NCORES):
        fs = slice(c * FPC, (c + 1) * FPC)
        in_maps.append({
            "xT": xT,
            "wqT": np.ascontiguousarray(wq_p[fs].T),   # [D, 256]
            "wkT": np.ascontiguousarray(wk_p[fs].T),
            "wvT": np.ascontiguousarray(wv[fs].T),
            "woT": np.ascontiguousarray(wo[:, fs].T),  # [256, D]
            "cosS": cosS,
            "sinS": sinS,
            "masks": m,
            "onesd": ones,
        })
    return in_maps


def _run(inputs, trace=False):
    from concourse.bass_utils import run_bass_kernel_spmd

    if "nc" not in _CACHE:
        _CACHE["nc"] = _build_nc()
    nc = _CACHE["nc"]

    in_maps = _host_prep(
        inputs["x"], inputs["wq"], inputs["wk"], inputs["wv"], inputs["wo"]
    )
    res = run_bass_kernel_spmd(nc, in_maps, list(range(NCORES)), trace=trace)
    acc = None
    for c in range(NCORES):
        part = res.results[c]["outp"]
        acc = part.copy() if acc is None else acc + part
    out = acc.reshape(B, S, D).astype(np.float32)
    return out, res


def kernel(**inputs) -> np.ndarray:
    out, _ = _run(inputs, trace=False)
    return out
